# revision 11
# baseline (speedup 1.0000x reference)
"""DeltaAttention Trainium2 kernel — 8-core SPMD via bass/Tile.

Math (per reference): 4 DeltaResidualBlocks (d_v=1) wrapped around MHA.
Because each delta block consumes its v_in only through the scalar
projection v_in @ dWv[i], the Wq/Wk/Wv/Wo matmuls collapse into single
extra columns of the dWk matmuls (precomputed on host), and attn@v
collapses to 2 output columns per head:
    n_h[q] = E_h[q,:] @ u_h,  r_h[q] = E_h[q,:] @ 1,  u_h = v_h @ w_h
    v3[q]  = sum_h n_h/r_h + const,   w = Wo @ dWv[3]
Sharding: 512 query tokens per core; k^T and u are AllGathered within
each 4-core batch group.

Perf structure:
  - all big matmuls in fp8 with perf_mode=DoubleRow (2 fp8 MACs/cell):
    delta k_proj matmuls use xT8/aug pairs over the contract dim; the
    E@[u,1] matmuls pair two key chunks.  dWk is host-scaled by 64 (k
    is L2-normalized, so any uniform scale on k_raw cancels exactly).
  - softmax exp is shifted by a global -8 (n/r is invariant to per-query
    shifts) so E fits fp8e5; half the exp tiles run on ACT (spline exp),
    half on DVE via a Schraudolph bit-trick: bits = max(ps,-C)+C
    converted to int8 and bitcast to fp8e5.  The Schraudolph log2-scale
    is folded into the fp8 q^T/k^T tiles.
  - LayerNorm statistics from precomputed moments (sum x / sum x^2 from
    host; k3 moments from the delta-3 pass); the g/b elementwise runs
    on GpSimd to keep DVE off the critical path.
"""

import os
from contextlib import ExitStack

import numpy as np
import ml_dtypes

import concourse.bass as bass
import concourse.mybir as mybir
import concourse.tile as tile
from concourse.bass_utils import run_bass_kernel_spmd
from concourse.masks import make_identity

dt = mybir.dt
AF = mybir.ActivationFunctionType
ALU = mybir.AluOpType
DR = mybir.MatmulPerfMode.DoubleRow
ts = bass.ts

N_CORES = 8
B, S, D, H = 2, 2048, 1024, 16
HD = D // H
TOK = (B * S) // N_CORES          # 512 query tokens per core
M4 = TOK // 128                   # 4 token chunks
K8 = D // 128                     # 8 feature chunks
NKC = S // 128                    # 16 key chunks per batch
EPS = 1e-8
LN_EPS = 1e-5

AUGS = 64.0                       # host scale on dWk (cancels via k-norm)
USC = 64.0                        # host scale on u columns
SHIFT = 8.0                       # global softmax shift (cancels in n/r)
SCHS = 4.0 / float(np.log(2.0))   # e5m2 quarter-steps per ln-unit
PRE = float(np.sqrt(0.125 * SCHS))  # folded into q^T and k^T each
EXP_SCALE = 0.125 / (PRE * PRE)   # == 1/SCHS; ACT exp scale on prescaled ps
C_SCH = float(os.environ.get("DELTA_CSCH", "13.734"))
C16 = 14771.43                    # bf16-bits Schraudolph constant (incl. -8 shift)

# extras matmul columns: [dbw0,vw0, dbw1,vw1, dbw2,vw2, Wu(16), Bu(16), dbw3]
W_EX = 39
EX_DBW = [0, 2, 4, 38]
EX_VW = [1, 3, 5]
EX_A = 6      # 6..22  = Wu * USC
EX_B = 22     # 22..38 = dWk2 @ Wu * AUGS * USC

LAST_RESULTS = None
_CACHE = {}


def _split_multi_waits(nc, max_waits=1):
    """walrus (CoreV3) only encodes one sync wait per instruction; Tile's
    final drain can carry several. Hoist extras onto preceding NoOps."""
    n_fixed = 0
    for f in nc.m.functions:
        for blk in f.blocks:
            new_insts = []
            for inst in blk.instructions:
                si = inst.sync_info
                waits = list(si.on_wait) if (si and si.on_wait) else []
                if len(waits) > max_waits:
                    head, tail = waits[:-max_waits], waits[-max_waits:]
                    for j, w in enumerate(head):
                        nop = mybir.InstNoOp(
                            name=f"{inst.name}_waitsplit_{j}",
                            engine=inst.engine,
                            ins=[],
                            outs=[],
                            sync_info=mybir.SyncInfo(on_wait=[w], on_update=[]),
                        )
                        nc.register_instruction(nop)
                        new_insts.append(nop)
                        n_fixed += 1
                    si.on_wait = tail
                new_insts.append(inst)
            blk.instructions[:] = new_insts
    return n_fixed


def _build_program():
    nc = bass.Bass(num_devices=N_CORES)

    x_t = nc.dram_tensor("x", [TOK, D], dt.float32, kind="ExternalInput")
    aug_t = [
        nc.dram_tensor(f"aug{i}", [128, K8 * D], dt.float8e4, kind="ExternalInput")
        for i in range(4)
    ]
    ex_t = nc.dram_tensor("ex", [D, W_EX], dt.float8e4, kind="ExternalInput")
    cvec_t = nc.dram_tensor("cvec", [128, 16], dt.float32, kind="ExternalInput")
    mxxx_t = nc.dram_tensor("mxxx", [128, 8], dt.float32, kind="ExternalInput")
    lng_t = nc.dram_tensor("lng", [128, D], dt.float32, kind="ExternalInput")
    lnb_t = nc.dram_tensor("lnb", [128, D], dt.float32, kind="ExternalInput")
    y_t = nc.dram_tensor("y", [TOK, D], dt.float32, kind="ExternalOutput")

    RG = [[0, 1, 2, 3], [4, 5, 6, 7]]

    with tile.TileContext(nc) as tc, ExitStack() as stack:
        const = stack.enter_context(tc.tile_pool(name="const", bufs=1))
        dram = stack.enter_context(tc.tile_pool(name="dram", bufs=1, space="DRAM"))
        big = stack.enter_context(tc.tile_pool(name="big", bufs=1))

        agk_in = dram.tile([D, TOK], dt.float8e4, tag="agk_in")
        agk_pc = [
            dram.tile([4 * 512, TOK], dt.float8e4, tag=f"agk_pc{j}", name=f"agk_pc{j}")
            for j in range(2)
        ]
        agu_in = dram.tile([TOK, H], dt.float8e4, tag="agu_in")
        agu_out = dram.tile([4 * TOK, H], dt.float8e4, tag="agu_out")

        ident_bf = const.tile([128, 128], dt.bfloat16, tag="ident_bf")
        make_identity(nc, ident_bf[:])
        ident_f32 = const.tile([128, 128], dt.float32, tag="ident_f32")
        make_identity(nc, ident_f32[:])
        cvec = const.tile([128, 16], dt.float32, tag="cvec")
        nc.sync.dma_start(cvec[:], cvec_t[:])
        mxxx = const.tile([128, 8], dt.float32, tag="mxxx")
        nc.sync.dma_start(mxxx[:], mxxx_t[:])
        lng = const.tile([128, D], dt.float32, tag="lng")
        lnb = const.tile([128, D], dt.float32, tag="lnb")

        # persistent data tiles
        x32 = [big.tile([128, D], dt.float32, tag=f"x32_{m}", name=f"x32_{m}") for m in range(M4)]
        xbf = [big.tile([128, D], dt.bfloat16, tag=f"xbf_{m}", name=f"xbf_{m}") for m in range(M4)]
        xT8 = big.tile([128, K8, TOK], dt.float8e4, tag="xT8")
        qT8 = big.tile([128, K8, TOK], dt.float8e4, tag="qT8")
        k3raw = [big.tile([128, D], dt.bfloat16, tag=f"k3_{m}", name=f"k3_{m}") for m in range(M4)]
        a3v = big.tile([128, M4], dt.float32, tag="a3v")
        b3v = big.tile([128, M4], dt.float32, tag="b3v")
        u8 = [big.tile([128, H], dt.float8e4, tag=f"u_{m}", name=f"u_{m}") for m in range(M4)]
        exsb = [big.tile([128, W_EX], dt.float32, tag=f"ex_{m}", name=f"ex_{m}") for m in range(M4)]
        v3acc = big.tile([128, M4], dt.float32, tag="v3acc")
        momk = big.tile([128, M4], dt.float32, tag="momk")
        momkk = big.tile([128, M4], dt.float32, tag="momkk")
        momxk = big.tile([128, M4], dt.float32, tag="momxk")
        cbig = big.tile([128, 2, TOK], dt.float32, tag="cbig")
        aug_sb = [
            big.tile([128, K8, D], dt.float8e4, tag=f"augsb_{i}", name=f"augsb_{i}")
            for i in range(4)
        ]

        nc.vector.memset(v3acc[:], 0.0)
        nc.vector.memset(cbig[:], C16)
        for m in range(M4):
            nc.sync.dma_start(x32[m][:], x_t[ts(m, 128), :])
            nc.scalar.copy(xbf[m][:], x32[m][:])
        # weight loads: delta-1 first (it runs first), delta-3 last
        for i in (1, 2, 0, 3):
            nc.sync.dma_start(
                aug_sb[i][:].rearrange("p c d -> p (c d)"), aug_t[i][:]
            )
        nc.sync.dma_start(lng[:], lng_t[:])
        nc.sync.dma_start(lnb[:], lnb_t[:])

        with (
            tc.tile_pool(name="qkpool", bufs=4) as qkpool,
            tc.tile_pool(name="scpool", bufs=24) as scpool,
            tc.tile_pool(name="scr", bufs=2) as scrpool,
            tc.tile_pool(name="ktloc", bufs=8) as ktlpool,
            tc.tile_pool(name="expool", bufs=8) as expool,
            tc.tile_pool(name="pp_proj", bufs=2, space="PSUM") as pp_proj,
            tc.tile_pool(name="pp_ex", bufs=2, space="PSUM") as pp_ex,
            tc.tile_pool(name="pp_t", bufs=2, space="PSUM") as pp_t,
        ):
            # x^T via PE transpose (bf16 in, fp8 out)
            for k in range(K8):
                pst = pp_t.tile([128, TOK], dt.bfloat16, tag="pst")
                for m in range(M4):
                    nc.tensor.transpose(
                        pst[:, ts(m, 128)], xbf[m][:, ts(k, 128)], ident_bf[:]
                    )
                nc.vector.tensor_copy(xT8[:, k, :], pst[:])

            # extras matmul: all betas / v-scalars / u components at once
            ext = [expool.tile([128, W_EX], dt.float8e4, tag="ext", name=f"ext_{k}") for k in range(K8)]
            for k in range(K8):
                nc.sync.dma_start(ext[k][:], ex_t[ts(k, 128), :])
            for m in range(M4):
                pse = pp_ex.tile([128, W_EX], dt.float32, tag="pse")
                for k in range(K8):
                    nc.tensor.matmul(
                        pse[:], xT8[:, k, ts(m, 128)], ext[k][:],
                        start=(k == 0), stop=(k == K8 - 1),
                    )
                nc.vector.tensor_copy(exsb[m][:], pse[:])

            qk_out = {}

            def scalar_chain(i, m, beta_src, kx, rnorm):
                """beta, rk, rr from per-chunk scalars. Returns (rk, rr)."""
                ez = scpool.tile([128, 1], dt.float32, tag="sc", name=f"ez_{i}_{m}")
                nc.scalar.activation(
                    ez[:], beta_src, AF.Exp, scale=-1.0 / USC, bias=cvec[:, i:i + 1]
                )
                ez1 = scpool.tile([128, 1], dt.float32, tag="sc", name=f"ez1_{i}_{m}")
                nc.vector.tensor_scalar_add(ez1[:], ez[:], 1.0)
                rsig = scpool.tile([128, 1], dt.float32, tag="sc", name=f"rs_{i}_{m}")
                nc.vector.reciprocal(rsig[:], ez1[:])
                rk = scpool.tile([128, 1], dt.float32, tag="sc", name=f"rk_{i}_{m}")
                nc.vector.tensor_scalar_mul(rk[:], kx, rnorm[:])
                rr = scpool.tile([128, 1], dt.float32, tag="sc", name=f"rr_{i}_{m}")
                nc.vector.tensor_scalar(rr[:], rsig[:], rnorm[:], 2.0, ALU.mult, ALU.mult)
                return rk, rr

            def rnorm_chain(i, m, ss):
                # 1/sqrt(ss) = exp(-0.5*ln(ss));  EPS=1e-8 is negligible
                lnv = scpool.tile([128, 1], dt.float32, tag="sc", name=f"lnv_{i}_{m}")
                nc.scalar.activation(lnv[:], ss, AF.Ln)
                rnorm = scpool.tile([128, 1], dt.float32, tag="sc", name=f"rn_{i}_{m}")
                nc.scalar.activation(rnorm[:], lnv[:], AF.Exp, scale=-0.5)
                return rnorm

            def proj_matmul(i, m, ps):
                for s0 in (0, 512):
                    for j in range(4):
                        nc.tensor.matmul(
                            ps[:, s0:s0 + 512],
                            xT8[:, 2 * j:2 * j + 2, ts(m, 128)],
                            aug_sb[i][:, 2 * j:2 * j + 2, s0:s0 + 512],
                            start=(j == 0), stop=(j == 3),
                            perf_mode=DR,
                        )

            def delta_block(i):
                """dWk matmul + delta elementwise for aug i on all 4 chunks."""
                outs = []
                for m in range(M4):
                    ps = pp_proj.tile([128, D], dt.float32, tag="ps_proj")
                    proj_matmul(i, m, ps)
                    ex = exsb[m]
                    scr = scrpool.tile([128, D], dt.bfloat16, tag="scr", name=f"scr_{i}_{m}")
                    ss = scpool.tile([128, 1], dt.float32, tag="sc", name=f"ss_{i}_{m}")
                    nc.scalar.activation(scr[:], ps[:], AF.Square, accum_out=ss[:])
                    kx = scpool.tile([128, 1], dt.float32, tag="sc", name=f"kx_{i}_{m}")
                    scr2 = scrpool.tile([128, D], dt.bfloat16, tag="scr", name=f"scr2_{i}_{m}")
                    nc.vector.scalar_tensor_tensor(
                        scr2[:], ps[:], 1.0, x32[m][:], ALU.mult, ALU.mult,
                        accum_out=kx[:],
                    )
                    rnorm = rnorm_chain(i, m, ss[:])
                    rk, rr = scalar_chain(i, m, ex[:, EX_DBW[i]:EX_DBW[i] + 1], kx[:], rnorm)
                    v = scpool.tile([128, 1], dt.float32, tag="sc", name=f"v_{i}_{m}")
                    nc.vector.tensor_scalar(
                        v[:], ex[:, EX_VW[i]:EX_VW[i] + 1], 1.0 / USC,
                        cvec[:, 4 + i:5 + i], ALU.mult, ALU.add,
                    )
                    dv = scpool.tile([128, 1], dt.float32, tag="sc", name=f"dv_{i}_{m}")
                    nc.vector.tensor_tensor(dv[:], v[:], rk[:], ALU.subtract)
                    s = scpool.tile([128, 1], dt.float32, tag="sc", name=f"s_{i}_{m}")
                    nc.vector.tensor_tensor(s[:], dv[:], rr[:], ALU.mult)
                    if i in (0, 1):
                        o = qkpool.tile([128, D], dt.bfloat16, tag="qk", name=f"qk_{i}_{m}")
                        nc.vector.scalar_tensor_tensor(
                            o[:], ps[:], s[:], x32[m][:], ALU.mult, ALU.add
                        )
                        outs.append(o)
                    else:
                        # i == 2: u*USC = B + s*A  (A/B pre-scaled in extras)
                        nc.vector.scalar_tensor_tensor(
                            u8[m][:], ex[:, EX_B:EX_B + H], s[:], ex[:, EX_A:EX_A + H],
                            ALU.mult, ALU.add,
                        )
                qk_out[i] = outs

            def delta3_chunk(m):
                """dWk3 matmul; elementwise + LN moments."""
                psd = pp_proj.tile([128, D], dt.float32, tag="ps_proj")
                proj_matmul(3, m, psd)
                mka = scpool.tile([128, 1], dt.float32, tag="sc", name=f"mka_{m}")
                mkb = scpool.tile([128, 1], dt.float32, tag="sc", name=f"mkb_{m}")
                nc.vector.tensor_scalar(
                    k3raw[m][:, 0:512], psd[:, 0:512], 1.0, 0.0, ALU.mult,
                    ALU.add, accum_out=mka[:],
                )
                nc.vector.tensor_scalar(
                    k3raw[m][:, 512:1024], psd[:, 512:1024], 1.0, 0.0, ALU.mult,
                    ALU.add, accum_out=mkb[:],
                )
                nc.vector.tensor_tensor(momk[:, m:m + 1], mka[:], mkb[:], ALU.add)
                scr = scrpool.tile([128, D], dt.bfloat16, tag="scr", name=f"sc3r_{m}")
                nc.scalar.activation(scr[:], psd[:], AF.Square, accum_out=momkk[:, m:m + 1])
                scr2 = scrpool.tile([128, D], dt.bfloat16, tag="scr", name=f"sc3r2_{m}")
                nc.vector.scalar_tensor_tensor(
                    scr2[:], psd[:], 1.0, x32[m][:], ALU.mult, ALU.mult,
                    accum_out=momxk[:, m:m + 1],
                )
                rnorm = rnorm_chain(3, m, momkk[:, m:m + 1])
                rk, rr = scalar_chain(3, m, exsb[m][:, EX_DBW[3]:EX_DBW[3] + 1], momxk[:, m:m + 1], rnorm)
                nc.vector.tensor_copy(a3v[:, m:m + 1], rr[:])
                nc.vector.tensor_tensor(b3v[:, m:m + 1], rr[:], rk[:], ALU.mult)

            def transpose_chunk(src_tiles, k, dst_ap, scale):
                pst = pp_t.tile([128, TOK], dt.bfloat16, tag="pst")
                for m in range(M4):
                    nc.tensor.transpose(
                        pst[:, ts(m, 128)], src_tiles[m][:, ts(k, 128)], ident_bf[:]
                    )
                if scale is None:
                    nc.vector.tensor_copy(dst_ap, pst[:])
                else:
                    nc.vector.tensor_scalar_mul(dst_ap, pst[:], scale)

            # ---- k path first so the AllGather starts early
            delta_block(1)
            ktloc = [ktlpool.tile([128, TOK], dt.float8e4, tag="ktloc", name=f"ktloc_{k}") for k in range(K8)]
            for k in range(K8):
                transpose_chunk(qk_out[1], k, ktloc[k][:], PRE)
                nc.sync.dma_start(agk_in[ts(k, 128), :], ktloc[k][:])
                if k == 3:
                    nc.gpsimd.collective_compute(
                        "AllGather", ALU.bypass, ins=[agk_in[0:512, :]],
                        outs=[agk_pc[0][:]], replica_groups=RG,
                    )
            delta_block(2)
            for m in range(M4):
                nc.sync.dma_start(agu_in[ts(m, 128), :], u8[m][:])
            nc.gpsimd.collective_compute(
                "AllGather", ALU.bypass, ins=[agu_in[:]], outs=[agu_out[:]],
                replica_groups=RG,
            )
            nc.gpsimd.collective_compute(
                "AllGather", ALU.bypass,
                ins=[agk_in[512:1024, :]], outs=[agk_pc[1][:]],
                replica_groups=RG,
            )
            delta_block(0)
            for k in range(K8):
                transpose_chunk(qk_out[0], k, qT8[:, k, :], PRE)
            for m in range(M4):
                delta3_chunk(m)

        # ---------------- attention ----------------
        with (
            tc.tile_pool(name="attn_sb", bufs=1) as attn_sb,
            tc.tile_pool(name="epool", bufs=3) as epool,
            tc.tile_pool(name="nrwp", bufs=2) as nrwp,
            tc.tile_pool(name="fin", bufs=2) as fin,
            tc.tile_pool(name="fing", bufs=2) as fing,
            tc.tile_pool(name="pp_sc", bufs=3, space="PSUM") as pp_sc,
            tc.tile_pool(name="pp_nr", bufs=1, space="PSUM") as pp_nr,
        ):
            kT = [attn_sb.tile([128, S], dt.float8e4, tag=f"kT_{k}", name=f"kTsb_{k}") for k in range(K8)]
            for k in range(K8):
                src = agk_pc[k // 4][:].rearrange("(c d) t -> d c t", c=4)[ts(k % 4, 128), :, :]
                dst = kT[k][:].rearrange("p (c t) -> p c t", c=4)
                nc.sync.dma_start(dst, src)
            u_all = attn_sb.tile([128, NKC, H], dt.float8e4, tag="u_all")
            nc.sync.dma_start(
                u_all[:], agu_out[:].rearrange("(kc p) h -> p kc h", p=128)
            )
            # stationary for E@[u,1]: [p, t, i, hp, c]; c: 0=u_even, 1=one, 2=u_odd
            u2 = attn_sb.tile([128, K8, 2, K8, 16], dt.float8e4, tag="u2")
            nc.vector.memset(u2[:], 0.0)
            nc.vector.memset(u2[:, :, :, :, 1], 1.0)
            uav = u_all[:].rearrange("p (t i) h -> p t i h", i=2)
            nc.vector.tensor_copy(u2[:, :, :, :, 0], uav[:, :, :, 0:H:2])
            nc.vector.tensor_copy(u2[:, :, :, :, 2], uav[:, :, :, 1:H:2])

            for hp in range(K8):         # 8 head pairs; pair hp = heads 2hp, 2hp+1
                nr_ps = pp_nr.tile([128, 2 * TOK], dt.float32, tag="nr")
                for t in range(K8):      # 8 key-chunk pairs
                    ps2 = [None, None]
                    for i in range(2):
                        kc = 2 * t + i
                        p2 = pp_sc.tile([128, 2, TOK], dt.float32, tag="sc2")
                        nc.tensor.matmul(
                            p2[:, 0, :], kT[hp][0:64, ts(kc, 128)], qT8[0:64, hp, :],
                            start=True, stop=True, tile_position=(0, 0),
                        )
                        nc.tensor.matmul(
                            p2[:, 1, :], kT[hp][64:128, ts(kc, 128)], qT8[64:128, hp, :],
                            start=True, stop=True, tile_position=(64, 0),
                        )
                        ps2[i] = p2
                    if t % 2 == 0:
                        # ACT pair: spline exp straight to fp8e5; DoubleRow E@[u,1]
                        E2 = epool.tile([128, 2, 2, TOK], dt.float8e5, tag="E2a")
                        for i in range(2):
                            nc.scalar.activation(
                                E2[:, i, :, :], ps2[i][:], AF.Exp,
                                scale=EXP_SCALE, bias=cvec[:, 8:9],
                            )
                        nc.tensor.matmul(
                            nr_ps[0:2, 0:TOK], u2[:, t, :, hp, 0:2], E2[:, :, 0, :],
                            start=(t == 0), stop=False,
                            perf_mode=DR, tile_position=(0, 0),
                        )
                        nc.tensor.matmul(
                            nr_ps[0:2, TOK:2 * TOK], u2[:, t, :, hp, 1:3], E2[:, :, 1, :],
                            start=(t == 0), stop=False,
                            perf_mode=DR, tile_position=(0, 0),
                        )
                    else:
                        # DVE pair: bf16-bits Schraudolph; plain bf16 E@[u,1]
                        E2w = epool.tile([128, 2, 2, TOK], dt.int16, tag="E2d")
                        for i in range(2):
                            nc.vector.scalar_tensor_tensor(
                                E2w[:, i, :, :], ps2[i][:], 32.0, cbig[:],
                                ALU.mult, ALU.add,
                            )
                        last = (t == K8 - 1)
                        for i in range(2):
                            nc.tensor.matmul(
                                nr_ps[0:2, 0:TOK], u2[:, t, i, hp, 0:2],
                                E2w[:, i, 0, :].bitcast(dt.bfloat16),
                                start=False, stop=(last and i == 1),
                                tile_position=(0, 0),
                            )
                            nc.tensor.matmul(
                                nr_ps[0:2, TOK:2 * TOK], u2[:, t, i, hp, 1:3],
                                E2w[:, i, 1, :].bitcast(dt.bfloat16),
                                start=False, stop=(last and i == 1),
                                tile_position=(0, 0),
                            )
                # stage the pair's n/r rows and fold into v3acc
                nrw = nrwp.tile([2, 2 * TOK], dt.float32, tag="nrw")
                nc.scalar.copy(nrw[:], nr_ps[0:2, :])
                psT = pp_sc.tile([128, 2, TOK], dt.float32, tag="sc2")
                for m in range(M4):
                    nc.tensor.transpose(
                        psT[:, 0, 4 * m:4 * m + 2], nrw[0:2, ts(m, 128)],
                        ident_f32[0:2, 0:2],
                    )
                    nc.tensor.transpose(
                        psT[:, 0, 4 * m + 2:4 * m + 4],
                        nrw[0:2, TOK + 128 * m:TOK + 128 * (m + 1)],
                        ident_f32[0:2, 0:2],
                    )
                nrT = fin.tile([128, 4, 4], dt.float32, tag="nrT", name=f"nrT_{hp}")
                nc.vector.tensor_copy(nrT[:], psT[:, 0, 0:16])
                # per m: (nA, rA, rB, nB)
                rec = fin.tile([128, 4, 2], dt.float32, tag="rec", name=f"rec_{hp}")
                nc.vector.reciprocal(rec[:], nrT[:, :, 1:3])
                prod = fin.tile([128, 4, 2], dt.float32, tag="prod", name=f"pr_{hp}")
                nc.vector.scalar_tensor_tensor(
                    prod[:], nrT[:, :, 0:4:3], 1.0 / USC, rec[:], ALU.mult, ALU.mult
                )
                pv = fin.tile([128, 4], dt.float32, tag="pv", name=f"pv_{hp}")
                nc.vector.tensor_reduce(pv[:], prod[:], axis=mybir.AxisListType.X, op=ALU.add)
                nc.vector.tensor_tensor(v3acc[:], v3acc[:], pv[:], ALU.add)

            # ---- final delta + layernorm (stats from precomputed moments)
            # all [128, M4] vectors: one chain for the 4 token chunks
            v3 = fin.tile([128, M4], dt.float32, tag="v3")
            nc.vector.tensor_scalar_add(v3[:], v3acc[:], cvec[:, 7:8])
            s3 = fin.tile([128, M4], dt.float32, tag="s3")
            nc.vector.tensor_tensor(s3[:], v3[:], a3v[:], ALU.mult)
            nc.vector.tensor_tensor(s3[:], s3[:], b3v[:], ALU.subtract)
            # mu = (sum_x + s3*sum_k)/D
            mu = fin.tile([128, M4], dt.float32, tag="mu")
            nc.vector.tensor_tensor(mu[:], s3[:], momk[:], ALU.mult)
            nc.vector.tensor_tensor(mu[:], mu[:], mxxx[:, 0:4], ALU.add)
            nc.vector.tensor_scalar_mul(mu[:], mu[:], 1.0 / D)
            # E[y^2] = (xx + 2 s3 xk + s3^2 kk)/D ; var = E[y^2] - mu^2
            t1 = fin.tile([128, M4], dt.float32, tag="t1")
            nc.vector.tensor_tensor(t1[:], s3[:], momkk[:], ALU.mult)
            t2 = fin.tile([128, M4], dt.float32, tag="t2")
            nc.vector.tensor_scalar(t2[:], momxk[:], 2.0, None, ALU.mult)
            nc.vector.tensor_tensor(t2[:], t2[:], t1[:], ALU.add)
            nc.vector.tensor_tensor(t2[:], t2[:], s3[:], ALU.mult)
            nc.vector.tensor_tensor(t2[:], t2[:], mxxx[:, 4:8], ALU.add)
            var = fin.tile([128, M4], dt.float32, tag="var")
            nc.vector.tensor_scalar_mul(var[:], t2[:], 1.0 / D)
            mu2 = fin.tile([128, M4], dt.float32, tag="mu2")
            nc.vector.tensor_tensor(mu2[:], mu[:], mu[:], ALU.mult)
            nc.vector.tensor_tensor(var[:], var[:], mu2[:], ALU.subtract)
            nc.vector.tensor_scalar_add(var[:], var[:], LN_EPS)
            lnv2 = fin.tile([128, M4], dt.float32, tag="lnv2")
            nc.scalar.activation(lnv2[:], var[:], AF.Ln)
            rstd = fin.tile([128, M4], dt.float32, tag="rstd")
            nc.scalar.activation(rstd[:], lnv2[:], AF.Exp, scale=-0.5)
            for m in range(M4):
                # yn = ((x + s3*k3raw) - mu)*rstd ; out = yn*g + b
                w1 = fing.tile([128, D], dt.float32, tag="w1", name=f"w1_{m}")
                nc.vector.scalar_tensor_tensor(
                    w1[:], k3raw[m][:], s3[:, m:m + 1], x32[m][:], ALU.mult, ALU.add
                )
                w2 = fing.tile([128, D], dt.float32, tag="w2", name=f"w2_{m}")
                nc.vector.tensor_scalar(
                    w2[:], w1[:], mu[:, m:m + 1], rstd[:, m:m + 1],
                    ALU.subtract, ALU.mult,
                )
                w3 = fing.tile([128, D], dt.float32, tag="w3", name=f"w3_{m}")
                nc.gpsimd.tensor_tensor(w3[:], w2[:], lng[:], ALU.mult)
                yg = fing.tile([128, D], dt.float32, tag="yg", name=f"yg_{m}")
                nc.gpsimd.tensor_tensor(yg[:], w3[:], lnb[:], ALU.add)
                nc.sync.dma_start(y_t[ts(m, 128), :], yg[:])

    _split_multi_waits(nc)
    nc.finalize()
    return nc


def _host_prep(inputs):
    """Precompute augmented weights and constants; returns per-core in_maps."""
    f32 = np.float32
    x = np.asarray(inputs["x"], f32)
    Wq, bq = np.asarray(inputs["Wq"], f32), np.asarray(inputs["bq"], f32)
    Wk, bk = np.asarray(inputs["Wk"], f32), np.asarray(inputs["bk"], f32)
    Wv, bv = np.asarray(inputs["Wv"], f32), np.asarray(inputs["bv"], f32)
    Wo, bo = np.asarray(inputs["Wo"], f32), np.asarray(inputs["bo"], f32)
    dWk, dbw = np.asarray(inputs["dWk"], f32), np.asarray(inputs["dbw"], f32)
    dbb, dWv = np.asarray(inputs["dbb"], f32), np.asarray(inputs["dWv"], f32)
    dbv = np.asarray(inputs["dbv"], f32)
    ln_g, ln_b = np.asarray(inputs["ln_g"], f32), np.asarray(inputs["ln_b"], f32)

    w = Wo @ dWv[3]                                   # (D,)
    Wu = np.zeros((D, H), f32)
    for h in range(H):
        Wu[h * HD:(h + 1) * HD, h] = w[h * HD:(h + 1) * HD]
    Bu = dWk[2] @ Wu                                  # (D, H)

    vw = [Wq @ dWv[0], Wk @ dWv[1], Wv @ dWv[2]]
    vc = [float(bq @ dWv[0] + dbv[0]), float(bk @ dWv[1] + dbv[1]),
          float(bv @ dWv[2] + dbv[2])]
    c3 = float(bo @ dWv[3] + dbv[3])

    e4 = ml_dtypes.float8_e4m3
    augs = [
        np.ascontiguousarray(
            (dWk[i] * AUGS).reshape(K8, 128, D).transpose(1, 0, 2).reshape(128, K8 * D)
        ).astype(e4)
        for i in range(4)
    ]
    ex = np.zeros((D, W_EX), f32)
    for i in range(4):
        ex[:, EX_DBW[i]] = dbw[i] * USC
    for i in range(3):
        ex[:, EX_VW[i]] = vw[i] * USC
    ex[:, EX_A:EX_A + H] = Wu * USC
    ex[:, EX_B:EX_B + H] = Bu * (AUGS * USC)
    ex = ex.astype(e4)

    cvec = np.zeros((128, 16), f32)
    for i in range(4):
        cvec[:, i] = -dbb[i]
    for i in range(3):
        cvec[:, 4 + i] = vc[i]
    cvec[:, 7] = c3
    cvec[:, 8] = -SHIFT
    cvec[:, 9] = -C_SCH

    lng = np.broadcast_to(ln_g[None, :], (128, D)).copy()
    lnb = np.broadcast_to(ln_b[None, :], (128, D)).copy()

    xf = x.reshape(B * S, D)
    mx_all = xf.sum(axis=1)                # (B*S,)
    xx_all = (xf * xf).sum(axis=1)
    in_maps = []
    for c in range(N_CORES):
        mxxx = np.zeros((128, 8), f32)
        for m in range(M4):
            sl = slice(c * TOK + m * 128, c * TOK + (m + 1) * 128)
            mxxx[:, m] = mx_all[sl]
            mxxx[:, 4 + m] = xx_all[sl]
        mp = {
            "x": np.ascontiguousarray(xf[c * TOK:(c + 1) * TOK]),
            "ex": ex, "cvec": cvec, "mxxx": mxxx, "lng": lng, "lnb": lnb,
        }
        for i in range(4):
            mp[f"aug{i}"] = augs[i]
        in_maps.append(mp)
    return in_maps


def kernel(**inputs):
    global LAST_RESULTS
    if "nc" not in _CACHE:
        _CACHE["nc"] = _build_program()
    nc = _CACHE["nc"]
    in_maps = _host_prep(inputs)
    res = run_bass_kernel_spmd(nc, in_maps, core_ids=list(range(N_CORES)))
    LAST_RESULTS = res
    out = np.concatenate(
        [res.results[c]["y"] for c in range(N_CORES)], axis=0
    ).reshape(B, S, D)
    return out.astype(np.float32)


# revision 12
# speedup vs baseline: 1.0861x; 1.0861x over previous
"""DeltaAttention Trainium2 kernel — 8-core SPMD via bass/Tile.

Math (per reference): 4 DeltaResidualBlocks (d_v=1) wrapped around MHA.
Because each delta block consumes its v_in only through the scalar
projection v_in @ dWv[i], the Wq/Wk/Wv/Wo matmuls collapse into single
extra columns of the dWk matmuls (precomputed on host), and attn@v
collapses to 2 output columns per head:
    n_h[q] = E_h[q,:] @ u_h,  r_h[q] = E_h[q,:] @ 1,  u_h = v_h @ w_h
    v3[q]  = sum_h n_h/r_h + const,   w = Wo @ dWv[3]
Sharding: 512 query tokens per core; k^T and u are AllGathered within
each 4-core batch group.

Perf structure:
  - all big matmuls in fp8 with perf_mode=DoubleRow (2 fp8 MACs/cell):
    delta k_proj matmuls use xT8/aug pairs over the contract dim; the
    E@[u,1] matmuls pair two key chunks.  dWk is host-scaled by 64 (k
    is L2-normalized, so any uniform scale on k_raw cancels exactly).
  - softmax exp is shifted by a global -8 (n/r is invariant to per-query
    shifts) so E fits fp8e5; half the exp tiles run on ACT (spline exp),
    half on DVE via a Schraudolph bit-trick: bits = max(ps,-C)+C
    converted to int8 and bitcast to fp8e5.  The Schraudolph log2-scale
    is folded into the fp8 q^T/k^T tiles.
  - LayerNorm statistics from precomputed moments (sum x / sum x^2 from
    host; k3 moments from the delta-3 pass); the g/b elementwise runs
    on GpSimd to keep DVE off the critical path.
"""

import os
from contextlib import ExitStack

import numpy as np
import ml_dtypes

import concourse.bass as bass
import concourse.mybir as mybir
import concourse.tile as tile
from concourse.bass_utils import run_bass_kernel_spmd
from concourse.masks import make_identity

dt = mybir.dt
AF = mybir.ActivationFunctionType
ALU = mybir.AluOpType
DR = mybir.MatmulPerfMode.DoubleRow
ts = bass.ts

N_CORES = 8
B, S, D, H = 2, 2048, 1024, 16
HD = D // H
TOK = (B * S) // N_CORES          # 512 query tokens per core
M4 = TOK // 128                   # 4 token chunks
K8 = D // 128                     # 8 feature chunks
NKC = S // 128                    # 16 key chunks per batch
EPS = 1e-8
LN_EPS = 1e-5

AUGS = 64.0                       # host scale on dWk (cancels via k-norm)
USC = 64.0                        # host scale on u columns
SHIFT = 8.0                       # global softmax shift (cancels in n/r)
SCHS = 4.0 / float(np.log(2.0))   # e5m2 quarter-steps per ln-unit
PRE = float(np.sqrt(0.125 * SCHS))  # folded into q^T and k^T each
EXP_SCALE = 0.125 / (PRE * PRE)   # == 1/SCHS; ACT exp scale on prescaled ps
C_SCH = float(os.environ.get("DELTA_CSCH", "13.734"))
C16 = 14771.43                    # bf16-bits Schraudolph constant (incl. -8 shift)

# extras matmul columns: [dbw0,vw0, dbw1,vw1, dbw2,vw2, Wu(16), Bu(16), dbw3]
W_EX = 39
EX_DBW = [0, 2, 4, 38]
EX_VW = [1, 3, 5]
EX_A = 6      # 6..22  = Wu * USC
EX_B = 22     # 22..38 = dWk2 @ Wu * AUGS * USC

LAST_RESULTS = None
_CACHE = {}


def _split_multi_waits(nc, max_waits=1):
    """walrus (CoreV3) only encodes one sync wait per instruction; Tile's
    final drain can carry several. Hoist extras onto preceding NoOps."""
    n_fixed = 0
    for f in nc.m.functions:
        for blk in f.blocks:
            new_insts = []
            for inst in blk.instructions:
                si = inst.sync_info
                waits = list(si.on_wait) if (si and si.on_wait) else []
                if len(waits) > max_waits:
                    head, tail = waits[:-max_waits], waits[-max_waits:]
                    for j, w in enumerate(head):
                        nop = mybir.InstNoOp(
                            name=f"{inst.name}_waitsplit_{j}",
                            engine=inst.engine,
                            ins=[],
                            outs=[],
                            sync_info=mybir.SyncInfo(on_wait=[w], on_update=[]),
                        )
                        nc.register_instruction(nop)
                        new_insts.append(nop)
                        n_fixed += 1
                    si.on_wait = tail
                new_insts.append(inst)
            blk.instructions[:] = new_insts
    return n_fixed


def _build_program():
    nc = bass.Bass(num_devices=N_CORES)

    x_t = nc.dram_tensor("x", [TOK, D], dt.float32, kind="ExternalInput")
    aug_t = [
        nc.dram_tensor(f"aug{i}", [128, K8 * D], dt.float8e4, kind="ExternalInput")
        for i in range(4)
    ]
    ex_t = nc.dram_tensor("ex", [D, W_EX], dt.float8e4, kind="ExternalInput")
    cvec_t = nc.dram_tensor("cvec", [128, 16], dt.float32, kind="ExternalInput")
    mxxx_t = nc.dram_tensor("mxxx", [128, 8], dt.float32, kind="ExternalInput")
    lng_t = nc.dram_tensor("lng", [128, D], dt.float32, kind="ExternalInput")
    lnb_t = nc.dram_tensor("lnb", [128, D], dt.float32, kind="ExternalInput")
    y_t = nc.dram_tensor("y", [TOK, D], dt.float32, kind="ExternalOutput")

    RG = [[0, 1, 2, 3], [4, 5, 6, 7]]

    with tile.TileContext(nc) as tc, ExitStack() as stack:
        const = stack.enter_context(tc.tile_pool(name="const", bufs=1))
        dram = stack.enter_context(tc.tile_pool(name="dram", bufs=1, space="DRAM"))
        big = stack.enter_context(tc.tile_pool(name="big", bufs=1))

        agk_in = dram.tile([D, TOK], dt.float8e4, tag="agk_in")
        agk_pc = [
            dram.tile([4 * 512, TOK], dt.float8e4, tag=f"agk_pc{j}", name=f"agk_pc{j}")
            for j in range(2)
        ]
        agu_in = dram.tile([TOK, H], dt.float8e4, tag="agu_in")
        agu_out = dram.tile([4 * TOK, H], dt.float8e4, tag="agu_out")

        ident_bf = const.tile([128, 128], dt.bfloat16, tag="ident_bf")
        make_identity(nc, ident_bf[:])
        ident_f32 = const.tile([128, 128], dt.float32, tag="ident_f32")
        make_identity(nc, ident_f32[:])
        cvec = const.tile([128, 16], dt.float32, tag="cvec")
        nc.sync.dma_start(cvec[:], cvec_t[:])
        mxxx = const.tile([128, 8], dt.float32, tag="mxxx")
        nc.sync.dma_start(mxxx[:], mxxx_t[:])
        lng = const.tile([128, D], dt.float32, tag="lng")
        lnb = const.tile([128, D], dt.float32, tag="lnb")

        # persistent data tiles
        x32 = [big.tile([128, D], dt.float32, tag=f"x32_{m}", name=f"x32_{m}") for m in range(M4)]
        xbf = [big.tile([128, D], dt.bfloat16, tag=f"xbf_{m}", name=f"xbf_{m}") for m in range(M4)]
        xT8 = big.tile([128, K8, TOK], dt.float8e4, tag="xT8")
        qT8 = big.tile([128, K8, TOK], dt.float8e4, tag="qT8")
        k3raw = [big.tile([128, D], dt.bfloat16, tag=f"k3_{m}", name=f"k3_{m}") for m in range(M4)]
        a3v = big.tile([128, M4], dt.float32, tag="a3v")
        b3v = big.tile([128, M4], dt.float32, tag="b3v")
        u8 = [big.tile([128, H], dt.float8e4, tag=f"u_{m}", name=f"u_{m}") for m in range(M4)]
        exsb = [big.tile([128, W_EX], dt.float32, tag=f"ex_{m}", name=f"ex_{m}") for m in range(M4)]
        v3acc = big.tile([128, M4], dt.float32, tag="v3acc")
        momk = big.tile([128, M4], dt.float32, tag="momk")
        momkk = big.tile([128, M4], dt.float32, tag="momkk")
        momxk = big.tile([128, M4], dt.float32, tag="momxk")
        cbig = big.tile([128, 2, TOK], dt.float32, tag="cbig")
        aug_sb = [
            big.tile([128, K8, D], dt.float8e4, tag=f"augsb_{i}", name=f"augsb_{i}")
            for i in range(4)
        ]

        nc.vector.memset(v3acc[:], 0.0)
        nc.vector.memset(cbig[:], C_SCH)
        for m in range(M4):
            nc.sync.dma_start(x32[m][:], x_t[ts(m, 128), :])
            nc.scalar.copy(xbf[m][:], x32[m][:])
        # weight loads: delta-1 first (it runs first), delta-3 last
        for i in (1, 2, 0, 3):
            nc.sync.dma_start(
                aug_sb[i][:].rearrange("p c d -> p (c d)"), aug_t[i][:]
            )
        nc.sync.dma_start(lng[:], lng_t[:])
        nc.sync.dma_start(lnb[:], lnb_t[:])

        with (
            tc.tile_pool(name="qkpool", bufs=4) as qkpool,
            tc.tile_pool(name="scpool", bufs=24) as scpool,
            tc.tile_pool(name="scr", bufs=2) as scrpool,
            tc.tile_pool(name="ktloc", bufs=8) as ktlpool,
            tc.tile_pool(name="expool", bufs=8) as expool,
            tc.tile_pool(name="pp_proj", bufs=2, space="PSUM") as pp_proj,
            tc.tile_pool(name="pp_ex", bufs=2, space="PSUM") as pp_ex,
            tc.tile_pool(name="pp_t", bufs=2, space="PSUM") as pp_t,
        ):
            # x^T via PE transpose (bf16 in, fp8 out)
            for k in range(K8):
                pst = pp_t.tile([128, TOK], dt.bfloat16, tag="pst")
                for m in range(M4):
                    nc.tensor.transpose(
                        pst[:, ts(m, 128)], xbf[m][:, ts(k, 128)], ident_bf[:]
                    )
                nc.vector.tensor_copy(xT8[:, k, :], pst[:])

            # extras matmul: all betas / v-scalars / u components at once
            ext = [expool.tile([128, W_EX], dt.float8e4, tag="ext", name=f"ext_{k}") for k in range(K8)]
            for k in range(K8):
                nc.sync.dma_start(ext[k][:], ex_t[ts(k, 128), :])
            for m in range(M4):
                pse = pp_ex.tile([128, W_EX], dt.float32, tag="pse")
                for k in range(K8):
                    nc.tensor.matmul(
                        pse[:], xT8[:, k, ts(m, 128)], ext[k][:],
                        start=(k == 0), stop=(k == K8 - 1),
                    )
                nc.vector.tensor_copy(exsb[m][:], pse[:])

            qk_out = {}

            def scalar_chain(i, m, beta_src, kx, rnorm):
                """beta, rk, rr from per-chunk scalars. Returns (rk, rr)."""
                ez = scpool.tile([128, 1], dt.float32, tag="sc", name=f"ez_{i}_{m}")
                nc.scalar.activation(
                    ez[:], beta_src, AF.Exp, scale=-1.0 / USC, bias=cvec[:, i:i + 1]
                )
                ez1 = scpool.tile([128, 1], dt.float32, tag="sc", name=f"ez1_{i}_{m}")
                nc.vector.tensor_scalar_add(ez1[:], ez[:], 1.0)
                rsig = scpool.tile([128, 1], dt.float32, tag="sc", name=f"rs_{i}_{m}")
                nc.vector.reciprocal(rsig[:], ez1[:])
                rk = scpool.tile([128, 1], dt.float32, tag="sc", name=f"rk_{i}_{m}")
                nc.vector.tensor_scalar_mul(rk[:], kx, rnorm[:])
                rr = scpool.tile([128, 1], dt.float32, tag="sc", name=f"rr_{i}_{m}")
                nc.vector.tensor_scalar(rr[:], rsig[:], rnorm[:], 2.0, ALU.mult, ALU.mult)
                return rk, rr

            def rnorm_chain(i, m, ss):
                # 1/sqrt(ss) = exp(-0.5*ln(ss));  EPS=1e-8 is negligible
                lnv = scpool.tile([128, 1], dt.float32, tag="sc", name=f"lnv_{i}_{m}")
                nc.scalar.activation(lnv[:], ss, AF.Ln)
                rnorm = scpool.tile([128, 1], dt.float32, tag="sc", name=f"rn_{i}_{m}")
                nc.scalar.activation(rnorm[:], lnv[:], AF.Exp, scale=-0.5)
                return rnorm

            def proj_matmul(i, m, ps):
                for s0 in (0, 512):
                    for j in range(4):
                        nc.tensor.matmul(
                            ps[:, s0:s0 + 512],
                            xT8[:, 2 * j:2 * j + 2, ts(m, 128)],
                            aug_sb[i][:, 2 * j:2 * j + 2, s0:s0 + 512],
                            start=(j == 0), stop=(j == 3),
                            perf_mode=DR,
                        )

            def delta_block(i):
                """dWk matmul + delta elementwise for aug i on all 4 chunks."""
                outs = []
                for m in range(M4):
                    ps = pp_proj.tile([128, D], dt.float32, tag="ps_proj")
                    proj_matmul(i, m, ps)
                    ex = exsb[m]
                    scr = scrpool.tile([128, D], dt.bfloat16, tag="scr", name=f"scr_{i}_{m}")
                    ss = scpool.tile([128, 1], dt.float32, tag="sc", name=f"ss_{i}_{m}")
                    nc.scalar.activation(scr[:], ps[:], AF.Square, accum_out=ss[:])
                    kx = scpool.tile([128, 1], dt.float32, tag="sc", name=f"kx_{i}_{m}")
                    scr2 = scrpool.tile([128, D], dt.bfloat16, tag="scr", name=f"scr2_{i}_{m}")
                    nc.vector.scalar_tensor_tensor(
                        scr2[:], ps[:], 1.0, x32[m][:], ALU.mult, ALU.mult,
                        accum_out=kx[:],
                    )
                    rnorm = rnorm_chain(i, m, ss[:])
                    rk, rr = scalar_chain(i, m, ex[:, EX_DBW[i]:EX_DBW[i] + 1], kx[:], rnorm)
                    v = scpool.tile([128, 1], dt.float32, tag="sc", name=f"v_{i}_{m}")
                    nc.vector.tensor_scalar(
                        v[:], ex[:, EX_VW[i]:EX_VW[i] + 1], 1.0 / USC,
                        cvec[:, 4 + i:5 + i], ALU.mult, ALU.add,
                    )
                    dv = scpool.tile([128, 1], dt.float32, tag="sc", name=f"dv_{i}_{m}")
                    nc.vector.tensor_tensor(dv[:], v[:], rk[:], ALU.subtract)
                    s = scpool.tile([128, 1], dt.float32, tag="sc", name=f"s_{i}_{m}")
                    nc.vector.tensor_tensor(s[:], dv[:], rr[:], ALU.mult)
                    if i in (0, 1):
                        o = qkpool.tile([128, D], dt.bfloat16, tag="qk", name=f"qk_{i}_{m}")
                        nc.vector.scalar_tensor_tensor(
                            o[:], ps[:], s[:], x32[m][:], ALU.mult, ALU.add
                        )
                        outs.append(o)
                    else:
                        # i == 2: u*USC = B + s*A  (A/B pre-scaled in extras)
                        nc.vector.scalar_tensor_tensor(
                            u8[m][:], ex[:, EX_B:EX_B + H], s[:], ex[:, EX_A:EX_A + H],
                            ALU.mult, ALU.add,
                        )
                qk_out[i] = outs

            def delta3_chunk(m):
                """dWk3 matmul; elementwise + LN moments."""
                psd = pp_proj.tile([128, D], dt.float32, tag="ps_proj")
                proj_matmul(3, m, psd)
                mka = scpool.tile([128, 1], dt.float32, tag="sc", name=f"mka_{m}")
                mkb = scpool.tile([128, 1], dt.float32, tag="sc", name=f"mkb_{m}")
                nc.vector.tensor_scalar(
                    k3raw[m][:, 0:512], psd[:, 0:512], 1.0, 0.0, ALU.mult,
                    ALU.add, accum_out=mka[:],
                )
                nc.vector.tensor_scalar(
                    k3raw[m][:, 512:1024], psd[:, 512:1024], 1.0, 0.0, ALU.mult,
                    ALU.add, accum_out=mkb[:],
                )
                nc.vector.tensor_tensor(momk[:, m:m + 1], mka[:], mkb[:], ALU.add)
                scr = scrpool.tile([128, D], dt.bfloat16, tag="scr", name=f"sc3r_{m}")
                nc.scalar.activation(scr[:], psd[:], AF.Square, accum_out=momkk[:, m:m + 1])
                scr2 = scrpool.tile([128, D], dt.bfloat16, tag="scr", name=f"sc3r2_{m}")
                nc.vector.scalar_tensor_tensor(
                    scr2[:], psd[:], 1.0, x32[m][:], ALU.mult, ALU.mult,
                    accum_out=momxk[:, m:m + 1],
                )
                rnorm = rnorm_chain(3, m, momkk[:, m:m + 1])
                rk, rr = scalar_chain(3, m, exsb[m][:, EX_DBW[3]:EX_DBW[3] + 1], momxk[:, m:m + 1], rnorm)
                nc.vector.tensor_copy(a3v[:, m:m + 1], rr[:])
                nc.vector.tensor_tensor(b3v[:, m:m + 1], rr[:], rk[:], ALU.mult)

            def transpose_chunk(src_tiles, k, dst_ap, scale):
                pst = pp_t.tile([128, TOK], dt.bfloat16, tag="pst")
                for m in range(M4):
                    nc.tensor.transpose(
                        pst[:, ts(m, 128)], src_tiles[m][:, ts(k, 128)], ident_bf[:]
                    )
                if scale is None:
                    nc.vector.tensor_copy(dst_ap, pst[:])
                else:
                    nc.vector.tensor_scalar_mul(dst_ap, pst[:], scale)

            # ---- k path first so the AllGather starts early
            delta_block(1)
            ktloc = [ktlpool.tile([128, TOK], dt.float8e4, tag="ktloc", name=f"ktloc_{k}") for k in range(K8)]
            for k in range(K8):
                transpose_chunk(qk_out[1], k, ktloc[k][:], PRE)
                nc.sync.dma_start(agk_in[ts(k, 128), :], ktloc[k][:])
                if k == 3:
                    nc.gpsimd.collective_compute(
                        "AllGather", ALU.bypass, ins=[agk_in[0:512, :]],
                        outs=[agk_pc[0][:]], replica_groups=RG,
                    )
            delta_block(2)
            for m in range(M4):
                nc.sync.dma_start(agu_in[ts(m, 128), :], u8[m][:])
            nc.gpsimd.collective_compute(
                "AllGather", ALU.bypass, ins=[agu_in[:]], outs=[agu_out[:]],
                replica_groups=RG,
            )
            nc.gpsimd.collective_compute(
                "AllGather", ALU.bypass,
                ins=[agk_in[512:1024, :]], outs=[agk_pc[1][:]],
                replica_groups=RG,
            )
            delta_block(0)
            for k in range(K8):
                transpose_chunk(qk_out[0], k, qT8[:, k, :], PRE)
            for m in range(M4):
                delta3_chunk(m)

        # ---------------- attention ----------------
        with (
            tc.tile_pool(name="attn_sb", bufs=1) as attn_sb,
            tc.tile_pool(name="epool", bufs=3) as epool,
            tc.tile_pool(name="nrwp", bufs=2) as nrwp,
            tc.tile_pool(name="fin", bufs=2) as fin,
            tc.tile_pool(name="fing", bufs=2) as fing,
            tc.tile_pool(name="pp_sc", bufs=3, space="PSUM") as pp_sc,
            tc.tile_pool(name="pp_nr", bufs=1, space="PSUM") as pp_nr,
        ):
            kT = [attn_sb.tile([128, S], dt.float8e4, tag=f"kT_{k}", name=f"kTsb_{k}") for k in range(K8)]
            for k in range(K8):
                src = agk_pc[k // 4][:].rearrange("(c d) t -> d c t", c=4)[ts(k % 4, 128), :, :]
                dst = kT[k][:].rearrange("p (c t) -> p c t", c=4)
                nc.sync.dma_start(dst, src)
            u_all = attn_sb.tile([128, NKC, H], dt.float8e4, tag="u_all")
            nc.sync.dma_start(
                u_all[:], agu_out[:].rearrange("(kc p) h -> p kc h", p=128)
            )
            # stationary for E@[u,1]: [p, t, i, hp, c]; c: 0=u_even, 1=one, 2=u_odd
            u2 = attn_sb.tile([128, K8, 2, K8, 16], dt.float8e4, tag="u2")
            nc.vector.memset(u2[:], 0.0)
            nc.vector.memset(u2[:, :, :, :, 1], 1.0)
            uav = u_all[:].rearrange("p (t i) h -> p t i h", i=2)
            nc.vector.tensor_copy(u2[:, :, :, :, 0], uav[:, :, :, 0:H:2])
            nc.vector.tensor_copy(u2[:, :, :, :, 2], uav[:, :, :, 1:H:2])

            for hp in range(K8):         # 8 head pairs; pair hp = heads 2hp, 2hp+1
                nr_ps = pp_nr.tile([128, 2 * TOK], dt.float32, tag="nr")
                for t in range(K8):      # 8 key-chunk pairs
                    ps2 = [None, None]
                    for i in range(2):
                        kc = 2 * t + i
                        p2 = pp_sc.tile([128, 2, TOK], dt.float32, tag="sc2")
                        nc.tensor.matmul(
                            p2[:, 0, :], kT[hp][0:64, ts(kc, 128)], qT8[0:64, hp, :],
                            start=True, stop=True, tile_position=(0, 0),
                        )
                        nc.tensor.matmul(
                            p2[:, 1, :], kT[hp][64:128, ts(kc, 128)], qT8[64:128, hp, :],
                            start=True, stop=True, tile_position=(64, 0),
                        )
                        ps2[i] = p2
                    E2 = epool.tile([128, 2, 2, TOK], dt.float8e5, tag="E2")
                    for i in range(2):
                        if (2 * t + i) % 4 == 3:
                            # DVE Schraudolph share (~25%): bits=max(ps,-C)+C -> int8
                            nc.vector.scalar_tensor_tensor(
                                E2[:, i, :, :].bitcast(dt.int8), ps2[i][:],
                                cvec[:, 9:10], cbig[:], ALU.max, ALU.add,
                            )
                        else:
                            nc.scalar.activation(
                                E2[:, i, :, :], ps2[i][:], AF.Exp,
                                scale=EXP_SCALE, bias=cvec[:, 8:9],
                            )
                    nc.tensor.matmul(
                        nr_ps[0:2, 0:TOK], u2[:, t, :, hp, 0:2], E2[:, :, 0, :],
                        start=(t == 0), stop=(t == K8 - 1),
                        perf_mode=DR, tile_position=(0, 0),
                    )
                    nc.tensor.matmul(
                        nr_ps[0:2, TOK:2 * TOK], u2[:, t, :, hp, 1:3], E2[:, :, 1, :],
                        start=(t == 0), stop=(t == K8 - 1),
                        perf_mode=DR, tile_position=(0, 0),
                    )
                # stage the pair's n/r rows and fold into v3acc
                nrw = nrwp.tile([2, 2 * TOK], dt.float32, tag="nrw")
                nc.scalar.copy(nrw[:], nr_ps[0:2, :])
                psT = pp_sc.tile([128, 2, TOK], dt.float32, tag="sc2")
                for m in range(M4):
                    nc.tensor.transpose(
                        psT[:, 0, 4 * m:4 * m + 2], nrw[0:2, ts(m, 128)],
                        ident_f32[0:2, 0:2],
                    )
                    nc.tensor.transpose(
                        psT[:, 0, 4 * m + 2:4 * m + 4],
                        nrw[0:2, TOK + 128 * m:TOK + 128 * (m + 1)],
                        ident_f32[0:2, 0:2],
                    )
                nrT = fin.tile([128, 4, 4], dt.float32, tag="nrT", name=f"nrT_{hp}")
                nc.vector.tensor_copy(nrT[:], psT[:, 0, 0:16])
                # per m: (nA, rA, rB, nB)
                rec = fin.tile([128, 4, 2], dt.float32, tag="rec", name=f"rec_{hp}")
                nc.vector.reciprocal(rec[:], nrT[:, :, 1:3])
                prod = fin.tile([128, 4, 2], dt.float32, tag="prod", name=f"pr_{hp}")
                nc.vector.scalar_tensor_tensor(
                    prod[:], nrT[:, :, 0:4:3], 1.0 / USC, rec[:], ALU.mult, ALU.mult
                )
                pv = fin.tile([128, 4], dt.float32, tag="pv", name=f"pv_{hp}")
                nc.vector.tensor_reduce(pv[:], prod[:], axis=mybir.AxisListType.X, op=ALU.add)
                nc.vector.tensor_tensor(v3acc[:], v3acc[:], pv[:], ALU.add)

            # ---- final delta + layernorm (stats from precomputed moments)
            # all [128, M4] vectors: one chain for the 4 token chunks
            v3 = fin.tile([128, M4], dt.float32, tag="v3")
            nc.vector.tensor_scalar_add(v3[:], v3acc[:], cvec[:, 7:8])
            s3 = fin.tile([128, M4], dt.float32, tag="s3")
            nc.vector.tensor_tensor(s3[:], v3[:], a3v[:], ALU.mult)
            nc.vector.tensor_tensor(s3[:], s3[:], b3v[:], ALU.subtract)
            # mu = (sum_x + s3*sum_k)/D
            mu = fin.tile([128, M4], dt.float32, tag="mu")
            nc.vector.tensor_tensor(mu[:], s3[:], momk[:], ALU.mult)
            nc.vector.tensor_tensor(mu[:], mu[:], mxxx[:, 0:4], ALU.add)
            nc.vector.tensor_scalar_mul(mu[:], mu[:], 1.0 / D)
            # E[y^2] = (xx + 2 s3 xk + s3^2 kk)/D ; var = E[y^2] - mu^2
            t1 = fin.tile([128, M4], dt.float32, tag="t1")
            nc.vector.tensor_tensor(t1[:], s3[:], momkk[:], ALU.mult)
            t2 = fin.tile([128, M4], dt.float32, tag="t2")
            nc.vector.tensor_scalar(t2[:], momxk[:], 2.0, None, ALU.mult)
            nc.vector.tensor_tensor(t2[:], t2[:], t1[:], ALU.add)
            nc.vector.tensor_tensor(t2[:], t2[:], s3[:], ALU.mult)
            nc.vector.tensor_tensor(t2[:], t2[:], mxxx[:, 4:8], ALU.add)
            var = fin.tile([128, M4], dt.float32, tag="var")
            nc.vector.tensor_scalar_mul(var[:], t2[:], 1.0 / D)
            mu2 = fin.tile([128, M4], dt.float32, tag="mu2")
            nc.vector.tensor_tensor(mu2[:], mu[:], mu[:], ALU.mult)
            nc.vector.tensor_tensor(var[:], var[:], mu2[:], ALU.subtract)
            nc.vector.tensor_scalar_add(var[:], var[:], LN_EPS)
            lnv2 = fin.tile([128, M4], dt.float32, tag="lnv2")
            nc.scalar.activation(lnv2[:], var[:], AF.Ln)
            rstd = fin.tile([128, M4], dt.float32, tag="rstd")
            nc.scalar.activation(rstd[:], lnv2[:], AF.Exp, scale=-0.5)
            for m in range(M4):
                # yn = ((x + s3*k3raw) - mu)*rstd ; out = yn*g + b
                w1 = fing.tile([128, D], dt.float32, tag="w1", name=f"w1_{m}")
                nc.vector.scalar_tensor_tensor(
                    w1[:], k3raw[m][:], s3[:, m:m + 1], x32[m][:], ALU.mult, ALU.add
                )
                w2 = fing.tile([128, D], dt.float32, tag="w2", name=f"w2_{m}")
                nc.vector.tensor_scalar(
                    w2[:], w1[:], mu[:, m:m + 1], rstd[:, m:m + 1],
                    ALU.subtract, ALU.mult,
                )
                w3 = fing.tile([128, D], dt.float32, tag="w3", name=f"w3_{m}")
                nc.gpsimd.tensor_tensor(w3[:], w2[:], lng[:], ALU.mult)
                yg = fing.tile([128, D], dt.float32, tag="yg", name=f"yg_{m}")
                nc.gpsimd.tensor_tensor(yg[:], w3[:], lnb[:], ALU.add)
                nc.sync.dma_start(y_t[ts(m, 128), :], yg[:])

    _split_multi_waits(nc)
    nc.finalize()
    return nc


def _host_prep(inputs):
    """Precompute augmented weights and constants; returns per-core in_maps."""
    f32 = np.float32
    x = np.asarray(inputs["x"], f32)
    Wq, bq = np.asarray(inputs["Wq"], f32), np.asarray(inputs["bq"], f32)
    Wk, bk = np.asarray(inputs["Wk"], f32), np.asarray(inputs["bk"], f32)
    Wv, bv = np.asarray(inputs["Wv"], f32), np.asarray(inputs["bv"], f32)
    Wo, bo = np.asarray(inputs["Wo"], f32), np.asarray(inputs["bo"], f32)
    dWk, dbw = np.asarray(inputs["dWk"], f32), np.asarray(inputs["dbw"], f32)
    dbb, dWv = np.asarray(inputs["dbb"], f32), np.asarray(inputs["dWv"], f32)
    dbv = np.asarray(inputs["dbv"], f32)
    ln_g, ln_b = np.asarray(inputs["ln_g"], f32), np.asarray(inputs["ln_b"], f32)

    w = Wo @ dWv[3]                                   # (D,)
    Wu = np.zeros((D, H), f32)
    for h in range(H):
        Wu[h * HD:(h + 1) * HD, h] = w[h * HD:(h + 1) * HD]
    Bu = dWk[2] @ Wu                                  # (D, H)

    vw = [Wq @ dWv[0], Wk @ dWv[1], Wv @ dWv[2]]
    vc = [float(bq @ dWv[0] + dbv[0]), float(bk @ dWv[1] + dbv[1]),
          float(bv @ dWv[2] + dbv[2])]
    c3 = float(bo @ dWv[3] + dbv[3])

    e4 = ml_dtypes.float8_e4m3
    augs = [
        np.ascontiguousarray(
            (dWk[i] * AUGS).reshape(K8, 128, D).transpose(1, 0, 2).reshape(128, K8 * D)
        ).astype(e4)
        for i in range(4)
    ]
    ex = np.zeros((D, W_EX), f32)
    for i in range(4):
        ex[:, EX_DBW[i]] = dbw[i] * USC
    for i in range(3):
        ex[:, EX_VW[i]] = vw[i] * USC
    ex[:, EX_A:EX_A + H] = Wu * USC
    ex[:, EX_B:EX_B + H] = Bu * (AUGS * USC)
    ex = ex.astype(e4)

    cvec = np.zeros((128, 16), f32)
    for i in range(4):
        cvec[:, i] = -dbb[i]
    for i in range(3):
        cvec[:, 4 + i] = vc[i]
    cvec[:, 7] = c3
    cvec[:, 8] = -SHIFT
    cvec[:, 9] = -C_SCH

    lng = np.broadcast_to(ln_g[None, :], (128, D)).copy()
    lnb = np.broadcast_to(ln_b[None, :], (128, D)).copy()

    xf = x.reshape(B * S, D)
    mx_all = xf.sum(axis=1)                # (B*S,)
    xx_all = (xf * xf).sum(axis=1)
    in_maps = []
    for c in range(N_CORES):
        mxxx = np.zeros((128, 8), f32)
        for m in range(M4):
            sl = slice(c * TOK + m * 128, c * TOK + (m + 1) * 128)
            mxxx[:, m] = mx_all[sl]
            mxxx[:, 4 + m] = xx_all[sl]
        mp = {
            "x": np.ascontiguousarray(xf[c * TOK:(c + 1) * TOK]),
            "ex": ex, "cvec": cvec, "mxxx": mxxx, "lng": lng, "lnb": lnb,
        }
        for i in range(4):
            mp[f"aug{i}"] = augs[i]
        in_maps.append(mp)
    return in_maps


def kernel(**inputs):
    global LAST_RESULTS
    if "nc" not in _CACHE:
        _CACHE["nc"] = _build_program()
    nc = _CACHE["nc"]
    in_maps = _host_prep(inputs)
    res = run_bass_kernel_spmd(nc, in_maps, core_ids=list(range(N_CORES)))
    LAST_RESULTS = res
    out = np.concatenate(
        [res.results[c]["y"] for c in range(N_CORES)], axis=0
    ).reshape(B, S, D)
    return out.astype(np.float32)
